# revision 20
# baseline (speedup 1.0000x reference)
"""Quantized matmul (uint4 groupwise dequant) on 8 Trainium2 NeuronCores.

Computes out = a_f32 @ W where W[k, n] = (q[k, n] - zeros[k//128, n]) * scales[k//128, n].

Sharding: tensor-parallel along N (output features). Each of the 8 cores gets
N_LOCAL = 512 columns of q/scales/zeros and the full `a` (replicated). Each
core dequantizes its W slice to fp16 once into SBUF, then runs a dense
fp16 matmul with fp32 PSUM accumulation.

Device kernel layout choices (all host-side prep is pure layout/sharding):
 - `a` is fed pre-transposed and tiled as aT[m_out, k_in, k_out*128 + m_in]
   so each [128, 4096] SBUF tile is one contiguous 1 MiB DMA and slices
   [:, k*128:(k+1)*128] are matmul lhsT tiles (K on partitions).
 - q values are 0..15, so the int32 container is narrowed to int8 on the
   host (lossless) to quarter its DMA cost.
 - zeros/scales arrive as ONE tiny [2, KT*NL] fp16 row-pair (64 KiB DRAM
   read). They are replicated across the 128 partitions with SBUF->SBUF
   partition-broadcast DMAs, so the 8.4 MB of replication writes never
   touch HBM and the ramp's HBM budget goes to q + aT instead.

Schedule: q chunks are interleaved ahead of the aT stream so the W pipeline
(q DMA -> zbc/sbc broadcast -> DVE sub+mul) is DVE-paced. The PE warms up
with 6 dummy matmuls (HAM un-throttles on a ~3.4us activity window), then
runs an 8-m-tile block-0 wavefront ordered by modeled operand arrival, then
m-outer/k-inner with inline epilogues. The last m-tile's epilogue is split
across ACT+DVE and two DMA queues to shorten the tail.
"""

import numpy as np

M, K, N = 4096, 4096, 4096
G = 128          # quant group size
P = 128          # partitions
NCORES = 8
NL = N // NCORES          # 512 output columns per core
KT = K // P               # 32 k tiles (== quant groups)
MT = M // P               # 32 m tiles
MBLK = 8                  # m-tiles in the wavefront block (8 PSUM banks)
AQ = 4                    # aT[0] is split into AQ sub-tiles
WARMUP = 6

_CACHE = {}


def _build_nc():
    import concourse.bacc as bacc
    import concourse.mybir as mybir
    import concourse.tile as tile
    from concourse.bass import ts

    f16 = mybir.dt.float16
    f32 = mybir.dt.float32
    i8 = mybir.dt.int8

    nc = bacc.Bacc("TRN2", target_bir_lowering=False, debug=False)

    aT = nc.dram_tensor("aT", [MT, P, K], f16, kind="ExternalInput").ap()
    q = nc.dram_tensor("q", [KT, P, NL], i8, kind="ExternalInput").ap()
    zsm = nc.dram_tensor("zsm", [1, KT * NL], f16, kind="ExternalInput").ap()
    ssm = nc.dram_tensor("ssm", [1, KT * NL], f16, kind="ExternalInput").ap()
    out = nc.dram_tensor("out", [MT, P, NL], f32, kind="ExternalOutput").ap()

    with tile.TileContext(nc) as tc:
        # chunk sizes (groups per chunk): small leading chunks for a short
        # path to W_0, larger later to bound trigger counts.
        CHUNKS = [1, 1, 2, 4, 4, 4, 4, 4, 4, 4]
        assert sum(CHUNKS) == KT

        # Availability model (us, relative to DMA-queue boot) used to order
        # the block-0 wavefront.
        RATE = 0.358     # MB/us of HBM bandwidth
        TRIG = 0.62      # us per DMA trigger on a queue
        DVE_G = 0.9      # us of engine work per dequantized group
        NSMALL = 4       # groups replicated by direct DRAM broadcast
        NBH = 14         # groups per big broadcast half

        avail_w = [0.0] * KT
        avail_a0 = [0.0] * AQ
        avail_at = [0.0] * MBLK

        with (
            tc.tile_pool(name="w", bufs=KT) as wpool,
            tc.tile_pool(name="zsb", bufs=3) as zbcpool,
            tc.tile_pool(name="ssb", bufs=3) as sbcpool,
            tc.tile_pool(name="zsbig", bufs=2) as bigpool,
            tc.tile_pool(name="qraw", bufs=3) as qpool,
            tc.tile_pool(name="deq", bufs=3) as dqpool,
            tc.tile_pool(name="a0", bufs=AQ) as a0pool,
            tc.tile_pool(name="at", bufs=8) as apool,
            tc.tile_pool(name="ot", bufs=3) as opool,
            tc.tile_pool(name="ps", bufs=MBLK, space="PSUM") as pspool,
        ):
            # --- PE warmup: dummy matmuls pull the HAM clock gate to 8/8.
            warm_in = dqpool.tile([P, NL], f16, name="warm_in", tag="d")
            nc.gpsimd.memset(warm_in[:], 0.0)
            warm_ps = pspool.tile([P, NL], f32, name="warm_ps", tag="ps")
            for i in range(WARMUP):
                nc.tensor.matmul(
                    warm_ps[:],
                    warm_in[:, 0:P],
                    warm_in[:],
                    start=(i == 0),
                    stop=(i == WARMUP - 1),
                )

            # --- emission bookkeeping for the availability model
            cum_mb = 0.0        # HBM bytes emitted so far
            eng_free = [0.0, 0.0]  # DVE / GpSimd dequant pipelines
            at0 = [None] * AQ
            ats0 = [None] * MBLK
            w_tiles = []
            zs_src = {}         # group -> (z ap, s ap, availability)

            # Groups 0..NSMALL-1: direct DRAM partition-broadcast (128x
            # read amplification, but tiny: NSMALL*256 KB total) for the
            # shortest path to W_0.
            for g in range(NSMALL):
                sl0 = slice(g * NL, (g + 1) * NL)
                zbc = zbcpool.tile([P, NL], f16, name=f"zbc{g}", tag="zb")
                nc.scalar.dma_start(zbc[:], zsm[:, sl0].partition_broadcast(P))
                sbc = sbcpool.tile([P, NL], f16, name=f"sbc{g}", tag="sb")
                nc.scalar.dma_start(sbc[:], ssm[:, sl0].partition_broadcast(P))
                cum_mb += 2 * (P * NL) * 2 / 1e6
                zs_src[g] = (zbc[:], sbc[:], cum_mb / RATE)

            # Groups NSMALL.. in two halves: DRAM -> partitions 0:16 with
            # 16x broadcast, then log-doubling SBUF->SBUF copies
            # (16->32->64->128). Replication writes stay off HBM.
            bc_free = 0.0
            for hh in range(2):
                g0 = NSMALL + hh * NBH
                sl0 = slice(g0 * NL, (g0 + NBH) * NL)
                zb = bigpool.tile([P, NBH * NL], f16, name=f"zbig{hh}", tag="zbig")
                sb = bigpool.tile([P, NBH * NL], f16, name=f"sbig{hh}", tag="sbig")
                nc.scalar.dma_start(
                    zb[0:16, :], zsm[:, sl0].partition_broadcast(16)
                )
                nc.scalar.dma_start(
                    sb[0:16, :], ssm[:, sl0].partition_broadcast(16)
                )
                cum_mb += 2 * (16 * NBH * NL) * 2 / 1e6
                bc_free = max(bc_free, cum_mb / RATE)
                for pp in (16, 32, 64):
                    nc.scalar.dma_start(zb[pp : 2 * pp, :], zb[0:pp, :])
                    nc.scalar.dma_start(sb[pp : 2 * pp, :], sb[0:pp, :])
                    bc_free += 2 * (pp * NBH * NL) * 2 / 1e6 / RATE
                for g in range(g0, g0 + NBH):
                    off = (g - g0) * NL
                    zs_src[g] = (
                        zb[:, off : off + NL],
                        sb[:, off : off + NL],
                        bc_free,
                    )

            def emit_at0_quarter(v):
                nonlocal cum_mb
                t = a0pool.tile([P, K // AQ], f16, name=f"at0q{v}", tag="a0")
                nc.sync.dma_start(t[:], aT[0][:, ts(v, K // AQ)])
                cum_mb += (P * K // AQ) * 2 / 1e6
                avail_a0[v] = cum_mb / RATE
                at0[v] = t

            def emit_at(mi):
                nonlocal cum_mb
                t = apool.tile([P, K], f16, name=f"at0_{mi}", tag="at")
                nc.sync.dma_start(t[:], aT[mi])
                cum_mb += (P * K) * 2 / 1e6
                avail_at[mi] = cum_mb / RATE
                ats0[mi] = t

            def emit_chunk(j, k_base, gpc):
                # q chunk DMA on the GpSimd (SWDGE) queue, then per-group
                # sub+mul alternating between DVE and GpSimd.
                nonlocal cum_mb
                qt = qpool.tile([P, gpc, NL], i8, name=f"qt{j}", tag="qt")
                nc.gpsimd.dma_start(
                    qt[:],
                    q[k_base : k_base + gpc].rearrange("g p n -> p g n"),
                )
                cum_mb += (P * gpc * NL) / 1e6
                q_arr = cum_mb / RATE

                for g in range(k_base, k_base + gpc):
                    zap, sap, zs_avail = zs_src[g]
                    ei = g % 2
                    eng = nc.vector if ei == 0 else nc.gpsimd
                    d = dqpool.tile([P, NL], f16, tag="d")
                    eng.tensor_sub(out=d[:], in0=qt[:, g - k_base, :], in1=zap)
                    wt = wpool.tile([P, NL], f16, tag="w")
                    eng.tensor_mul(out=wt[:], in0=d[:], in1=sap)
                    w_tiles.append(wt)
                    eng_free[ei] = max(eng_free[ei], q_arr, zs_avail) + DVE_G
                    avail_w[g] = eng_free[ei]

            # --- HBM stream emission: q chunks interleaved ahead of aT.
            k_base = 0
            for j, gpc in enumerate(CHUNKS):
                emit_chunk(j, k_base, gpc)
                k_base += gpc
                if j == 2:
                    for v in range(AQ):
                        emit_at0_quarter(v)
                elif j >= 3 and j - 2 < MBLK:
                    emit_at(j - 2)

            def lhsT(mi, k):
                if mi == 0:
                    return at0[k * AQ // KT][:, ts(k % (KT // AQ), P)]
                return ats0[mi][:, ts(k, P)]

            def avail_lhs(mi, k):
                return avail_a0[k * AQ // KT] if mi == 0 else avail_at[mi]

            # Block 0: emit (mi, k) matmuls in estimated-availability order.
            pss = [
                pspool.tile([P, NL], f32, name=f"ps0_{i}", tag="ps")
                for i in range(MBLK)
            ]
            # per-m prefix-max so each m's k-stream stays in k-order (the
            # k==0 matmul carries start=True and must execute first).
            keys = {}
            for mi in range(MBLK):
                run = 0.0
                for k in range(KT):
                    run = max(run, avail_lhs(mi, k), avail_w[k])
                    keys[(mi, k)] = run
            order = sorted(
                ((mi, k) for mi in range(MBLK) for k in range(KT)),
                key=lambda t: (keys[t], t[0], t[1]),
            )
            for mi, k in order:
                nc.tensor.matmul(
                    pss[mi][:],
                    lhsT(mi, k),
                    w_tiles[k][:],
                    start=(k == 0),
                    stop=(k == KT - 1),
                )
            for mi in range(MBLK):
                ot = opool.tile([P, NL], f32)
                nc.scalar.copy(ot[:], pss[mi][:])
                nc.scalar.dma_start(out[mi], ot[:])

            # Remaining m-tiles: m-outer, k-inner, inline epilogue.
            for m in range(MBLK, MT):
                at = apool.tile([P, K], f16, name=f"at_{m}", tag="at")
                nc.sync.dma_start(at[:], aT[m])
                ps = pspool.tile([P, NL], f32, name=f"ps_{m}", tag="ps")
                for k in range(KT):
                    nc.tensor.matmul(
                        ps[:],
                        at[:, ts(k, P)],
                        w_tiles[k][:],
                        start=(k == 0),
                        stop=(k == KT - 1),
                    )
                if m < MT - 1:
                    ot = opool.tile([P, NL], f32)
                    nc.scalar.copy(ot[:], ps[:])
                    nc.scalar.dma_start(out[m], ot[:])
                else:
                    # tail: split the last epilogue across ACT+DVE and two
                    # DMA queues.
                    h = NL // 2
                    ota = opool.tile([P, h], f32)
                    otb = opool.tile([P, h], f32)
                    nc.scalar.copy(ota[:], ps[:, :h])
                    nc.vector.tensor_copy(otb[:], ps[:, h:])
                    nc.scalar.dma_start(out[m][:, :h], ota[:])
                    nc.sync.dma_start(out[m][:, h:], otb[:])

    nc.compile()
    return nc


def _shard_inputs(a, q_weight, scales, zeros):
    """Host-side shard/layout. Pure slicing, transposition and replication."""
    # aT[m_out, k_in, k_out*128 + m_in] = a[m_out*128 + m_in, k_out*128 + k_in]
    aT = np.ascontiguousarray(
        a.reshape(MT, P, KT, P).transpose(0, 3, 2, 1)
    ).reshape(MT, P, K)
    # q values are 0..15: int8 container is lossless.
    q8 = q_weight.astype(np.int8)

    in_maps = []
    for c in range(NCORES):
        sl = slice(c * NL, (c + 1) * NL)
        q_c = np.ascontiguousarray(q8[:, sl]).reshape(KT, P, NL)
        z_c = np.ascontiguousarray(zeros[:, sl]).reshape(1, KT * NL)
        s_c = np.ascontiguousarray(scales[:, sl]).reshape(1, KT * NL)
        in_maps.append({"aT": aT, "q": q_c, "zsm": z_c, "ssm": s_c})
    return in_maps


def _run(inputs, trace=False):
    from concourse import bass_utils

    if "nc" not in _CACHE:
        _CACHE["nc"] = _build_nc()
    nc = _CACHE["nc"]

    a = np.asarray(inputs["a"], dtype=np.float16)
    q_weight = np.asarray(inputs["q_weight"], dtype=np.int32)
    scales = np.asarray(inputs["scales"], dtype=np.float16)
    zeros = np.asarray(inputs["zeros"], dtype=np.float16)

    in_maps = _shard_inputs(a, q_weight, scales, zeros)
    res = bass_utils.run_bass_kernel_spmd(
        nc, in_maps, core_ids=list(range(NCORES)), trace=trace
    )

    out = np.empty((M, N), dtype=np.float32)
    for c in range(NCORES):
        out[:, c * NL : (c + 1) * NL] = res.results[c]["out"].reshape(M, NL)
    return out, res


def kernel(**inputs) -> np.ndarray:
    out, _ = _run(inputs, trace=False)
    return out


# revision 25
# speedup vs baseline: 1.1215x; 1.1215x over previous
"""Quantized matmul (uint4 groupwise dequant) on 8 Trainium2 NeuronCores.

Computes out = a_f32 @ W where W[k, n] = (q[k, n] - zeros[k//128, n]) * scales[k//128, n].

Sharding: tensor-parallel along N (output features). Each of the 8 cores gets
N_LOCAL = 512 columns of q/scales/zeros and the full `a` (replicated). Each
core dequantizes its W slice to fp16 once into SBUF, then runs a dense
fp16 matmul with fp32 PSUM accumulation.

Device kernel layout choices (all host-side prep is pure layout/sharding):
 - `a` is fed pre-transposed and tiled as aT[m_out, k_in, k_out*128 + m_in]
   so each [128, 4096] SBUF tile is one contiguous 1 MiB DMA and slices
   [:, k*128:(k+1)*128] are matmul lhsT tiles (K on partitions).
 - q values are 0..15, so the int32 container is narrowed to int8 on the
   host (lossless) to quarter its DMA cost; the DVE subtract consumes the
   int8 operand directly (int8 - fp16 -> fp16 in one op).
 - scales/zeros come in as [32, 512] slices; both are broadcast across the
   128 partitions on-device with chunked stride-0 DRAM->SBUF DMAs.

Schedule: the PE warms up with 6 dummy matmuls (the HAM clock gate needs
~3.4us of activity to reach 8/8 = 2.4 GHz), then runs a single
availability-ordered wavefront covering m-tiles 0..14: aT quarters for
m0/m1 give the PE work in the first microseconds, and the extension to 15
m-tiles (PSUM banks recycled with explicit ordering keys) gives the
in-order PE stream enough backlog to absorb the W-dequant trickle without
going idle. Remaining m-tiles run m-outer/k-inner with inline epilogues;
the last epilogue is split across ACT+DVE and two DMA queues.
"""

import numpy as np

M, K, N = 4096, 4096, 4096
G = 128          # quant group size
P = 128          # partitions
NCORES = 8
NL = N // NCORES          # 512 output columns per core
KT = K // P               # 32 k tiles (== quant groups)
MT = M // P               # 32 m tiles
NQM = 2                   # m-tiles loaded as quarters (m0, m1)
NFULL = 6                 # early full m-tiles (m2..m7)
WAVE = 15                 # m-tiles in the availability-sorted wavefront
AQ = 4                    # quarters per quartered m-tile
WARMUP = 6

_CACHE = {}


def _build_nc():
    import concourse.bacc as bacc
    import concourse.mybir as mybir
    import concourse.tile as tile
    from concourse.bass import ts

    f16 = mybir.dt.float16
    f32 = mybir.dt.float32
    i8 = mybir.dt.int8

    nc = bacc.Bacc("TRN2", target_bir_lowering=False, debug=False)

    aT = nc.dram_tensor("aT", [MT, P, K], f16, kind="ExternalInput").ap()
    q = nc.dram_tensor("q", [KT, P, NL], i8, kind="ExternalInput").ap()
    zsm = nc.dram_tensor("zsm", [1, KT * NL], f16, kind="ExternalInput").ap()
    ssm = nc.dram_tensor("ssm", [1, KT * NL], f16, kind="ExternalInput").ap()
    out = nc.dram_tensor("out", [MT, P, NL], f32, kind="ExternalOutput").ap()

    with tile.TileContext(nc) as tc:
        CHUNKS = [1, 1, 1, 1, 2, 2, 4, 4, 4, 4, 4, 4]
        assert sum(CHUNKS) == KT

        # Availability model (us, relative to DMA boot) used to order the
        # wavefront: cumulative emitted HBM bytes over ~0.358 MB/us plus
        # the serial DVE dequant pipeline.
        RATE = 0.358
        DVE_G = 0.95

        avail_w = [0.0] * KT
        avail_aq = {}             # (mi, quarter) -> ready time, mi < NQM
        avail_at = [0.0] * WAVE   # full-tile ready times

        with (
            tc.tile_pool(name="w", bufs=KT) as wpool,
            tc.tile_pool(name="zsb", bufs=3) as zsbpool,
            tc.tile_pool(name="qraw", bufs=6) as qpool,
            tc.tile_pool(name="deq", bufs=4) as dqpool,
            tc.tile_pool(name="a0", bufs=NQM * AQ) as a0pool,
            tc.tile_pool(name="atb", bufs=NFULL) as bpool,
            tc.tile_pool(name="at", bufs=7) as apool,
            tc.tile_pool(name="ot", bufs=2) as opool,
            tc.tile_pool(name="ps", bufs=8, space="PSUM") as pspool,
        ):
            cum_mb = 0.0
            dve_free = 0.0
            aqt = {}              # (mi, v) -> quarter tile
            ats = [None] * WAVE   # full tiles (mi >= NQM)
            w_tiles = []

            # PE warm-up: dummy matmuls pull the HAM clock gate to 8/8
            # before real operands arrive.
            warm_in = dqpool.tile([P, NL], f16, name="warm_in", tag="d")
            nc.gpsimd.memset(warm_in[:], 0.0)
            warm_ps = pspool.tile([P, NL], f32, name="warm_ps", tag="ps")
            for i in range(WARMUP):
                nc.tensor.matmul(
                    warm_ps[:],
                    warm_in[:, 0:P],
                    warm_in[:],
                    start=(i == 0),
                    stop=(i == WARMUP - 1),
                )

            def emit_quarter(mi, v):
                nonlocal cum_mb
                t = a0pool.tile([P, K // AQ], f16, name=f"aq{mi}_{v}", tag="a0")
                nc.sync.dma_start(t[:], aT[mi][:, ts(v, K // AQ)])
                cum_mb += (P * K // AQ) * 2 / 1e6
                avail_aq[(mi, v)] = cum_mb / RATE
                aqt[(mi, v)] = t

            def emit_at(mi):
                nonlocal cum_mb
                pool = bpool if mi < NQM + NFULL else apool
                t = pool.tile([P, K], f16, name=f"at_{mi}", tag="at")
                nc.sync.dma_start(t[:], aT[mi])
                cum_mb += (P * K) * 2 / 1e6
                avail_at[mi] = cum_mb / RATE
                ats[mi] = t

            def emit_chunk(j, k_base, gpc):
                # Broadcasts issue from the Scalar sequencer and q loads
                # from GpSimd (SWDGE) so the ~0.6 us/DMA trigger cost isn't
                # serialized on the Sync sequencer with the aT loads.
                nonlocal cum_mb, dve_free
                zbc = zsbpool.tile([P, gpc * NL], f16, name=f"zbc{j}", tag="zb")
                nc.scalar.dma_start(
                    zbc[:],
                    zsm[:, k_base * NL : (k_base + gpc) * NL].partition_broadcast(P),
                )
                sbc = zsbpool.tile([P, gpc * NL], f16, name=f"sbc{j}", tag="sb")
                nc.scalar.dma_start(
                    sbc[:],
                    ssm[:, k_base * NL : (k_base + gpc) * NL].partition_broadcast(P),
                )
                cum_mb += 2 * (P * gpc * NL) * 2 / 1e6
                qt = qpool.tile([P, gpc, NL], i8, name=f"qt{j}", tag="qt")
                nc.gpsimd.dma_start(
                    qt[:],
                    q[k_base : k_base + gpc].rearrange("g p n -> p g n"),
                )
                cum_mb += (P * gpc * NL) / 1e6
                for g in range(gpc):
                    k = k_base + g
                    d = dqpool.tile([P, NL], f16, tag="d")
                    nc.vector.tensor_sub(
                        out=d[:], in0=qt[:, g, :], in1=zbc[:, ts(g, NL)]
                    )
                    wt = wpool.tile([P, NL], f16, tag="w")
                    nc.vector.tensor_mul(out=wt[:], in0=d[:], in1=sbc[:, ts(g, NL)])
                    w_tiles.append(wt)
                    dve_free = max(dve_free, cum_mb / RATE) + DVE_G
                    avail_w[k] = dve_free

            # Emission order: two tiny chunks lead (shortest path to W_0),
            # quarters of m0/m1 interleaved with more small chunks, then
            # the W pipeline with early aT tiles spread between chunks,
            # then the extension tiles.
            plan = [("c", 0), ("c", 1)]
            for v in range(AQ):
                plan += [("q", 0, v), ("q", 1, v)]
                if v < 3:
                    plan.append(("c", 2 + v))
            plan.append(("c", 5))
            ai = NQM
            for j in range(6, len(CHUNKS)):
                plan += [("a", ai), ("c", j)]
                ai += 1
            while ai < WAVE:
                plan.append(("a", ai))
                ai += 1

            kbases = [0]
            for gpc in CHUNKS:
                kbases.append(kbases[-1] + gpc)
            for item in plan:
                if item[0] == "c":
                    j = item[1]
                    emit_chunk(j, kbases[j], CHUNKS[j])
                elif item[0] == "q":
                    emit_quarter(item[1], item[2])
                else:
                    emit_at(item[1])

            def lhsT(mi, k):
                if mi < NQM:
                    return aqt[(mi, k * AQ // KT)][:, ts(k % (KT // AQ), P)]
                return ats[mi][:, ts(k, P)]

            def avail_lhs(mi, k):
                if mi < NQM:
                    return avail_aq[(mi, k * AQ // KT)]
                return avail_at[mi]

            # Wavefront over m0..WAVE-1, ordered by modeled availability.
            # Keys are prefix-maxed per m (k==0 carries start=True and must
            # go first) and chained across PSUM-bank reuse: tile i of the
            # pool cycle shares a bank with tile i-8, so its matmuls must
            # be emitted after the earlier tile's accumulation finished.
            pss = [
                pspool.tile([P, NL], f32, name=f"ps0_{i}", tag="ps")
                for i in range(WAVE)
            ]
            keys = {}
            for mi in range(WAVE):
                run = 0.0
                if mi >= 7:
                    # bank shared with pss[mi-8] (warm_ps offsets by one)
                    run = keys[(mi - 8, KT - 1)] if mi >= 8 else 0.0
                if mi == 7:
                    run = 0.0  # shares with warm_ps, free after warmup
                for k in range(KT):
                    run = max(run, avail_lhs(mi, k), avail_w[k])
                    keys[(mi, k)] = run
            order = sorted(
                ((mi, k) for mi in range(WAVE) for k in range(KT)),
                key=lambda t: (keys[t], t[0], t[1]),
            )
            for mi, k in order:
                nc.tensor.matmul(
                    pss[mi][:],
                    lhsT(mi, k),
                    w_tiles[k][:],
                    start=(k == 0),
                    stop=(k == KT - 1),
                )
            # Epilogues in completion order so the scalar queue drains the
            # PSUM banks in the order the wave finishes them.
            for mi in sorted(range(WAVE), key=lambda m: keys[(m, KT - 1)]):
                ot = opool.tile([P, NL], f32)
                nc.scalar.copy(ot[:], pss[mi][:])
                nc.scalar.dma_start(out[mi], ot[:])

            # Remaining m-tiles: m-outer, k-inner, inline epilogue.
            for m in range(WAVE, MT):
                at = apool.tile([P, K], f16, name=f"at_{m}", tag="at")
                nc.sync.dma_start(at[:], aT[m])
                ps = pspool.tile([P, NL], f32, name=f"ps_{m}", tag="ps")
                for k in range(KT):
                    nc.tensor.matmul(
                        ps[:],
                        at[:, ts(k, P)],
                        w_tiles[k][:],
                        start=(k == 0),
                        stop=(k == KT - 1),
                    )
                if m < MT - 1:
                    ot = opool.tile([P, NL], f32)
                    nc.scalar.copy(ot[:], ps[:])
                    nc.scalar.dma_start(out[m], ot[:])
                else:
                    # tail: split the last epilogue across ACT+DVE and two
                    # DMA queues.
                    h = NL // 2
                    ota = opool.tile([P, h], f32)
                    otb = opool.tile([P, h], f32)
                    nc.scalar.copy(ota[:], ps[:, :h])
                    nc.vector.tensor_copy(otb[:], ps[:, h:])
                    nc.scalar.dma_start(out[m][:, :h], ota[:])
                    nc.sync.dma_start(out[m][:, h:], otb[:])

    nc.compile()
    return nc


def _shard_inputs(a, q_weight, scales, zeros):
    """Host-side shard/layout. Pure slicing, transposition and replication."""
    # aT[m_out, k_in, k_out*128 + m_in] = a[m_out*128 + m_in, k_out*128 + k_in]
    aT = np.ascontiguousarray(
        a.reshape(MT, P, KT, P).transpose(0, 3, 2, 1)
    ).reshape(MT, P, K)
    # q values are 0..15: int8 container is lossless.
    q8 = q_weight.astype(np.int8)

    in_maps = []
    for c in range(NCORES):
        sl = slice(c * NL, (c + 1) * NL)
        q_c = np.ascontiguousarray(q8[:, sl]).reshape(KT, P, NL)
        z_c = np.ascontiguousarray(zeros[:, sl]).reshape(1, KT * NL)
        s_c = np.ascontiguousarray(scales[:, sl]).reshape(1, KT * NL)
        in_maps.append({"aT": aT, "q": q_c, "zsm": z_c, "ssm": s_c})
    return in_maps


def _run(inputs, trace=False):
    from concourse import bass_utils

    if "nc" not in _CACHE:
        _CACHE["nc"] = _build_nc()
    nc = _CACHE["nc"]

    a = np.asarray(inputs["a"], dtype=np.float16)
    q_weight = np.asarray(inputs["q_weight"], dtype=np.int32)
    scales = np.asarray(inputs["scales"], dtype=np.float16)
    zeros = np.asarray(inputs["zeros"], dtype=np.float16)

    in_maps = _shard_inputs(a, q_weight, scales, zeros)
    res = bass_utils.run_bass_kernel_spmd(
        nc, in_maps, core_ids=list(range(NCORES)), trace=trace
    )

    out = np.empty((M, N), dtype=np.float32)
    for c in range(NCORES):
        out[:, c * NL : (c + 1) * NL] = res.results[c]["out"].reshape(M, NL)
    return out, res


def kernel(**inputs) -> np.ndarray:
    out, _ = _run(inputs, trace=False)
    return out
